# revision 48
# baseline (speedup 1.0000x reference)
"""CQT magnitude kernel for Trainium2 (8 NeuronCores, Bass/Tile).

Strategy (v4)
-------------
C[k, n] = sum_l xpad[n*HOP + l] * kernel[k, l], tiled over 128-wide
l-chunks.  Core q owns chunks c = 8s+q (m0 = bins 0-127) and
c = 247+8s+q (m1 = bins 128-251); host sums the 8 per-core partials.

 * Contiguous rhs streams: core q only touches xpad chunks of one
   residue class mod 4 per band, so xi is packed [128, v, 4 tracks]
   and the rhs walk [[4,F],[1,4]] is sequential.  Measured 0.4375
   ns/col (1 col/cycle @ 2.4GHz) vs 0.90 for strided.
 * re/im interleaved weight columns (col 2t = kr bin t, 2t+1 = ki),
   bins split lo/hi at 64 so chunks with M<=64 need one matmul.
 * Diagonal groups: for edge chunks with tiny M, several slots are
   packed into ONE matmul.  Slot s0+d contributes rows at a frame
   shift of 2d (c = 8s+q, hop = 4 chunks), so one rhs sweep over the
   union interval serves them all; each group is a self-contained
   start&stop matmul whose rectangle is flushed separately and summed
   (shifted) on the host.  18 edge + 12 small slots -> 10 groups,
   ~6000 streamed cols saved.
 * 8 PSUM accumulators (m0/m1 x lo/hi x frame-half); m1 runs last so
   banks 4-7 double as group scratch during the m0 pass.
 * The PE clock ramps for ~5.8us after first activity: zero-weight
   ramp-hold matmuls run until data lands, then INIT + groups (cheap
   or LDWEIGHTS-bound work) fill the remaining ramp window.
 * Input DMAs are gated: the critical kt0+xa transfers get the full
   DMA bandwidth (later issues wait on a 1-col reader), landing ~1.3us
   earlier.
 * Flush pipelining: A-hi flushes under A-rest, A-lo under the m1
   pass, tail = m1 banks only, with copies and out-DMA split across
   engines/queues.  Output staged in bf16 (host re-sums in f32).
"""

import numpy as np

# ---- problem constants (hardcoded per contract) ----
SR = 44100
BPO = 36
KBINS = 252
FMIN = 32.70319566257483
QF = 1.0 / (2.0 ** (1.0 / BPO) - 1.0)
SR_B, SR_TR, SR_T = 2, 2, 65536
NTRACKS = SR_B * SR_TR            # 4
L = 69376                          # filterbank window length
HOP = 512
PCH = 128
NCH = L // PCH                     # 542 l-chunks
NF = 1 + SR_T // HOP               # 129 frames
NCORES = 8
M1C0 = 247                         # first m1 chunk
NS0 = 68                           # m0 slots per core
NS1 = 6                            # m1 slots per core (48 chunks)
J_VALID_LO, J_VALID_HI = 271, 782  # nonzero xpad chunk range (inclusive)
XPAD_CH = 1056
FH = 65                            # frame-half boundary: fh0=[0,64], fh1=[65,128]
VA0, VA = 64, 134                  # xiA v-window (v = (j - q)/4)
VB0, VB = 64, 133                  # xiB v-window (v = (j - q - 3)/4)
N_PRE = 16                         # zero-weight PE ramp-hold matmuls
S0_INIT = 33                       # m0 init slot (full frame coverage)
S1_INIT = 2                        # m1 init slot (forced full coverage)

# diagonal slot groups (sum of active rows <= 128, psum cols <= 512)
G_E = [[0, 1, 2, 3, 4, 5, 6], [7, 8], [59, 60, 61, 62, 63],
       [64, 65], [66, 67]]
G_R = [[9, 10, 11], [12, 13], [52, 53], [54, 55], [56, 57, 58]]
G_H = [[23, 24, 25], [26, 27], [39, 40], [41, 42, 43]]   # hi-bin groups

# bank ids: 0=A_lo0 1=A_lo1 2=A_hi0 3=A_hi1 4=B_lo0 5=B_lo1 6=B_hi0 7=B_hi1
_BANK_COLS = [260, 256, 260, 256, 260, 256, 260, 256]
_GBANKS = [4, 5, 6, 7, 4, 5, 6, 7, 4, 5,    # lo groups (during m0)
           6, 7, 4, 5]                       # hi groups (after m1 flush)


def _bank_of(m, kind, fh):
    return (4 if m == 1 else 0) + 2 * kind + fh


def _fh_windows(n0, n1):
    out = []
    if n0 < FH:
        out.append((0, n0, min(n1, FH - 1)))
    if n1 >= FH:
        out.append((1, max(n0, FH), n1))
    return out


def _build_tables():
    freqs = FMIN * 2.0 ** (np.arange(KBINS) / BPO)
    lens = QF * SR / freqs
    lo = np.floor((L // 2 - lens / 2) / PCH).astype(int)
    hi = np.ceil((L // 2 + lens / 2) / PCH).astype(int)
    m0c = np.zeros(NCH + 8, int)
    m1c = np.zeros(NCH + 8, int)
    for k in range(128):
        m0c[lo[k] : hi[k]] = np.maximum(m0c[lo[k] : hi[k]], k + 1)
    for k in range(128, KBINS):
        m1c[lo[k] : hi[k]] = np.maximum(m1c[lo[k] : hi[k]], k - 127)
    m0s = [max(m0c[8 * s + q] for q in range(8)) for s in range(NS0)]
    m1s = [max(m1c[M1C0 + 8 * s + q] for q in range(8)) for s in range(NS1)]

    def nrng(cl, ch):
        return max(0, -(-(J_VALID_LO - ch) // 4)), min(
            NF - 1, (J_VALID_HI - cl) // 4
        )

    f0 = [nrng(8 * s, 8 * s + 7) for s in range(NS0)]
    f1 = [nrng(M1C0 + 8 * s, M1C0 + 8 * s + 7) for s in range(NS1)]

    # groups: (s0, [(s, Mrows)], cols, lo_v, hi_v, binoff)
    groups = []
    for grp in G_E + G_R:
        s0 = grp[0]
        sm = [(s, int(m0s[s])) for s in grp]
        cols = sum(2 * M for _, M in sm)
        lo_v = min(f0[s][0] + 2 * (s - s0) for s in grp)
        hi_v = max(f0[s][1] + 2 * (s - s0) for s in grp)
        groups.append((s0, sm, cols, lo_v, hi_v, 0))
    for grp in G_H:
        s0 = grp[0]
        sm = [(s, int(m0s[s]) - 64) for s in grp]
        cols = sum(2 * M for _, M in sm)
        lo_v = min(f0[s][0] + 2 * (s - s0) for s in grp)
        hi_v = max(f0[s][1] + 2 * (s - s0) for s in grp)
        groups.append((s0, sm, cols, lo_v, hi_v, 64))

    gslots = set(s for g in G_E + G_R for s in g)
    ghslots = set(s for g in G_H for s in g)
    hi_slots = [s for s in range(NS0) if m0s[s] > 64 and s != S0_INIT]
    hi_sing = [s for s in hi_slots if s not in ghslots]
    a_reg = [s for s in range(9, 59)
             if s not in hi_slots and s != S0_INIT and s not in gslots]
    b_order = [2, 1, 4, 0, 5]

    def ne(m, s, kind):
        ms = m1s[s] if m == 1 else m0s[s]
        fw = f1[s] if m == 1 else f0[s]
        cols = 2 * (ms - 64) if kind == 1 else 2 * min(ms, 64)
        return ('n', m, s, kind, cols, fw[0], fw[1])

    # ops in emission order: INIT0 | E-groups | A-lo (lo entries + G_R) |
    # INIT1+B (m1, its flushes hide under the hi section) | A-hi (tail =
    # A-hi flush only)
    ops = [
        ('n', 0, S0_INIT, 0, 128, 0, NF - 1),
        ('n', 0, S0_INIT, 1, 128, 0, NF - 1),
    ]
    for gi in range(len(G_E)):
        ops.append(('g', gi))
    for s in hi_slots:
        ops.append(ne(0, s, 0))
    # a_reg with G_R groups interleaved
    ri = [0, 3, 6, 10, 13, len(a_reg)]
    for j in range(5):
        ops.append(('g', len(G_E) + j))
        for s in a_reg[ri[j] : ri[j + 1]]:
            ops.append(ne(0, s, 0))
    a_lo_end = len(ops)
    ib0 = len(ops)
    # m1 init = slot 3 itself (full coverage forced; hi cols padded)
    ops.append(('n', 1, 3, 0, 128, 0, NF - 1))
    ops.append(('n', 1, 3, 1, 128, 0, NF - 1))
    for s in b_order:
        ops.append(ne(1, s, 0))
        if m1s[s] > 64:
            ops.append(ne(1, s, 1))
    b_end = len(ops)
    # A-hi section: hi groups early (their flushes overlap the singles)
    nge = len(G_E) + len(G_R)
    ops.append(('g', nge + 0))
    for s in hi_sing[0:2]:
        ops.append(ne(0, s, 1))
    ops.append(('g', nge + 1))
    for s in hi_sing[2:4]:
        ops.append(ne(0, s, 1))
    ops.append(('g', nge + 2))
    for s in hi_sing[4:7]:
        ops.append(ne(0, s, 1))
    ops.append(('g', nge + 3))
    for s in hi_sing[7:]:
        ops.append(ne(0, s, 1))

    starts = (0, 1, ib0, ib0 + 1)
    bhi_end = ib0 + 4        # after s3 lo/hi + s2 lo/hi

    def opcols(op):
        return groups[op[1]][2] if op[0] == 'g' else op[4]

    offs = np.cumsum([0] + [opcols(op) for op in ops])
    # kt DMA groups: INIT0+E | A-lo x3 (sized to DMA arrival) | INIT1+B
    # | A-hi
    a0 = 2 + len(G_E)
    t1 = int(offs[a0]) + 800
    t2 = int(offs[a0]) + 2400
    g2 = int(np.searchsorted(offs, t1))
    g3 = int(np.searchsorted(offs, t2))
    gsplits = [0, a0, g2, g3, ib0, b_end, len(ops)]
    # st columns for group rectangles, appended after the 8 banks
    boff = np.cumsum([0] + _BANK_COLS)
    gst = []
    p = int(boff[-1])
    for (s0, sm, cols, lo_v, hi_v, binoff) in groups:
        gst.append(p)
        p += 4 * (hi_v - lo_v + 1)
    return ops, groups, gst, p, offs, gsplits, (
        starts, a_lo_end, ib0, bhi_end, b_end)


(_OPS, _GROUPS, _GST, _OUTC, _KTOFF, _GSPL,
 (_STARTS, _A_LO_END, _B0, _BHI_END, _B_END)) = _build_tables()
_NG = len(_GSPL) - 1
_GCOLS = [int(_KTOFF[_GSPL[g + 1]] - _KTOFF[_GSPL[g]]) for g in range(_NG)]


def _last_writer_per_bank():
    last = {}
    for ei, op in enumerate(_OPS):
        if op[0] != 'n':
            continue
        _, m, s, kind, cols, n0, n1 = op
        for fh, a, b in _fh_windows(n0, n1):
            last[_bank_of(m, kind, fh)] = ei
    return last


_LAST = _last_writer_per_bank()

_PROG = None


def _build_program():
    import concourse.bass as bass
    import concourse.mybir as mybir
    from concourse import bacc
    from concourse.tile import TileContext

    f32 = mybir.dt.float32
    bf16 = mybir.dt.bfloat16

    nc = bacc.Bacc(None, name="cqt_spmd4")
    kt_d = [
        nc.dram_tensor(f"kt{g}", [128, _GCOLS[g]], bf16, kind="ExternalInput")
        for g in range(_NG)
    ]
    xa_d = nc.dram_tensor("xa", [128, VA, 4], bf16, kind="ExternalInput")
    xb_d = nc.dram_tensor("xb", [128, VB, 4], bf16, kind="ExternalInput")
    out_d = nc.dram_tensor("out", [128, _OUTC], bf16, kind="ExternalOutput")

    with TileContext(nc) as tc:
        with (
            tc.tile_pool(name="xp", bufs=1) as xp,
            tc.tile_pool(name="ktp", bufs=1) as ktp,
            tc.tile_pool(name="wp", bufs=1) as wp,
            tc.tile_pool(name="accp", bufs=1, space="PSUM") as accp,
        ):
            # nonzero filler operands: zero-valued warmup matmuls toggle few
            # bits, so the power-managed clock ramp may not credit them as
            # activity; mixed values make the array switch for real
            wtile = wp.tile([128, 128], bf16, tag="wt", name="wt")
            nc.vector.memset(wtile, 1.0)
            nc.vector.memset(wtile[:, 0:2], -0.625)

            xa_t = xp.tile([128, VA, 4], bf16, tag="xa", name="xa")
            xb_t = xp.tile([128, VB, 4], bf16, tag="xb", name="xb")
            kt_t = [
                ktp.tile([128, _GCOLS[g]], bf16, tag=f"kt{g}", name=f"kt{g}")
                for g in range(_NG)
            ]
            # critical inputs (kt-init0+E, xa) first
            nc.scalar.dma_start(out=kt_t[0], in_=kt_d[0][:, :])
            nc.gpsimd.dma_start(out=xa_t, in_=xa_d[:, :, :])
            nc.scalar.dma_start(out=kt_t[1], in_=kt_d[1][:, :])
            nc.gpsimd.dma_start(out=kt_t[2], in_=kt_d[2][:, :])
            nc.scalar.dma_start(out=kt_t[3], in_=kt_d[3][:, :])
            nc.gpsimd.dma_start(out=kt_t[4], in_=kt_d[4][:, :])
            nc.scalar.dma_start(out=xb_t, in_=xb_d[:, :, :])
            nc.gpsimd.dma_start(out=kt_t[5], in_=kt_d[5][:, :])

            accs = [
                accp.tile([128, 512], f32, tag=f"acc{b}", name=f"acc{b}")
                for b in range(8)
            ]

            def zrhs(cols):
                return bass.AP(
                    tensor=wtile.tensor,
                    offset=wtile.offset,
                    ap=[wtile.ap[0], [0, cols // 4], [1, 4]],
                )

            for _ in range(N_PRE - 2):
                nc.tensor.matmul(
                    accs[0][:128, :260], wtile[:, :128], zrhs(260),
                    start=True, stop=True,
                )
            for _ in range(8):   # fine-grained tail: smaller data-wait quantum
                nc.tensor.matmul(
                    accs[0][:128, :64], wtile[:, :128], zrhs(64),
                    start=True, stop=True,
                )

            st = wp.tile([128, _OUTC], bf16, tag="st", name="st")
            boff = np.cumsum([0] + _BANK_COLS)

            def flush(banks, eng_cycle, dma_engs=None):
                for i, b in enumerate(banks):
                    eng = eng_cycle[i % len(eng_cycle)]
                    src = accs[b][:128, : _BANK_COLS[b]]
                    dst = st[:, int(boff[b]) : int(boff[b + 1])]
                    if hasattr(eng, "tensor_copy"):
                        eng.tensor_copy(dst, src)
                    else:
                        eng.copy(dst, src)
                lo = int(boff[banks[0]])
                hi = int(boff[banks[-1] + 1])
                dma_engs = dma_engs or [nc.gpsimd]
                n = len(dma_engs)
                cuts = [lo + (hi - lo) * i // n for i in range(n + 1)]
                for i, eng in enumerate(dma_engs):
                    eng.dma_start(
                        out=out_d[:, cuts[i] : cuts[i + 1]],
                        in_=st[:, cuts[i] : cuts[i + 1]],
                    )

            g = 0
            gcopy = 0
            for ei, op in enumerate(_OPS):
                while ei >= _GSPL[g + 1]:
                    g += 1
                off = int(_KTOFF[ei] - _KTOFF[_GSPL[g]])
                if op[0] == 'g':
                    gi = op[1]
                    s0, sm, cols, lo_v, hi_v, _bo = _GROUPS[gi]
                    ln = hi_v - lo_v + 1
                    bk = _GBANKS[gi]
                    lhsT = kt_t[g][:, off : off + cols]
                    rhs = bass.AP(
                        tensor=xa_t.tensor,
                        offset=xa_t.offset + (2 * s0 + lo_v - VA0) * 4,
                        ap=[xa_t.ap[0], [4, ln], [1, 4]],
                    )
                    nc.tensor.matmul(
                        accs[bk][:cols, : 4 * ln], lhsT, rhs,
                        start=True, stop=True,
                    )
                    ceng = [nc.vector, nc.scalar][gcopy % 2]
                    gcopy += 1
                    dst = st[:cols, _GST[gi] : _GST[gi] + 4 * ln]
                    src = accs[bk][:cols, : 4 * ln]
                    if hasattr(ceng, "tensor_copy"):
                        ceng.tensor_copy(dst, src)
                    else:
                        ceng.copy(dst, src)
                    nc.gpsimd.dma_start(
                        out=out_d[:cols, _GST[gi] : _GST[gi] + 4 * ln],
                        in_=st[:cols, _GST[gi] : _GST[gi] + 4 * ln],
                    )
                    continue
                _, m, s, kind, cols, n0, n1 = op
                lhsT = kt_t[g][:, off : off + cols]
                xi = xb_t if m == 1 else xa_t
                vbase = (61 + 2 * s - VB0) if m == 1 else (2 * s - VA0)
                for fh, a, b in _fh_windows(n0, n1):
                    F = b - a + 1
                    rhs = bass.AP(
                        tensor=xi.tensor,
                        offset=xi.offset + (vbase + a) * 4,
                        ap=[xi.ap[0], [4, F], [1, 4]],
                    )
                    bk = _bank_of(m, kind, fh)
                    fb = 0 if fh == 0 else FH
                    out = accs[bk][:cols, 4 * (a - fb) : 4 * (b + 1 - fb)]
                    nc.tensor.matmul(
                        out, lhsT, rhs,
                        start=(ei in _STARTS), stop=(_LAST[bk] == ei),
                    )
                if ei == _A_LO_END - 1:     # end of m0 lo: flush under B+hi
                    flush([0, 1], [nc.vector, nc.scalar],
                          [nc.gpsimd, nc.sync])
                if ei == _BHI_END - 1:      # m1 hi banks complete
                    flush([6, 7], [nc.vector, nc.scalar])
                if ei == _B_END - 1:        # m1 lo done: flush under A-hi
                    flush([4, 5], [nc.vector, nc.scalar],
                          [nc.gpsimd, nc.sync])
            # tail: A-hi banks; pipelined quarter-pieces (copy then its DMA)
            for i in range(4):
                b = 2 + i // 2
                h = _BANK_COLS[b] // 2
                c0 = int(boff[b]) + (i % 2) * h
                w = h if i % 2 == 0 else _BANK_COLS[b] - h
                eng = [nc.vector, nc.scalar][i % 2]
                src = accs[b][:128, (i % 2) * h : (i % 2) * h + w]
                dst = st[:, c0 : c0 + w]
                if hasattr(eng, "tensor_copy"):
                    eng.tensor_copy(dst, src)
                else:
                    eng.copy(dst, src)
                deng = [nc.gpsimd, nc.sync][i % 2]
                deng.dma_start(out=out_d[:, c0 : c0 + w], in_=st[:, c0 : c0 + w])
    nc.finalize()
    _dedupe_ldweights(nc)
    return nc


def _dedupe_ldweights(nc):
    """Drop back-to-back InstLdweights with identical weights APs."""
    for fn in nc.m.functions:
        for bb in fn.blocks:
            insts = list(bb.instructions)
            keep = []
            prev_key = None
            for inst in insts:
                if type(inst).__name__ == 'InstLdweights':
                    key = str(inst.ins[0])
                    si = inst.sync_info
                    clean = not si or (
                        len(si.on_wait) == 0 and len(si.on_update) == 0
                    )
                    if key == prev_key and clean:
                        continue
                    prev_key = key
                keep.append(inst)
            if len(keep) != len(insts):
                bb.instructions = keep


def _pack_inputs(x, kr, ki):
    import ml_dtypes

    bf16 = ml_dtypes.bfloat16
    xf = np.ascontiguousarray(
        np.asarray(x, dtype=np.float32).reshape(NTRACKS, SR_T)
    )
    kr = np.asarray(kr, dtype=np.float32)
    ki = np.asarray(ki, dtype=np.float32)

    xpad = np.zeros((NTRACKS, XPAD_CH * PCH), np.float32)
    xpad[:, L // 2 : L // 2 + SR_T] = xf
    xch = xpad.reshape(NTRACKS, XPAD_CH, PCH)      # [t, j, p]

    in_maps = []
    for q in range(NCORES):
        ja = q + 4 * (VA0 + np.arange(VA))
        jb = (q + 3) + 4 * (VB0 + np.arange(VB))
        xa = np.ascontiguousarray(
            xch[:, ja, :].transpose(2, 1, 0).astype(bf16)
        )
        xb = np.ascontiguousarray(
            xch[:, jb, :].transpose(2, 1, 0).astype(bf16)
        )

        def wblock(blk, c, binoff, nb):
            if c >= NCH:
                return
            sl = slice(c * PCH, (c + 1) * PCH)
            nb = min(nb, KBINS - binoff)
            blk[:, 0 : 2 * nb : 2] = kr[binoff : binoff + nb, sl].T
            blk[:, 1 : 2 * nb : 2] = ki[binoff : binoff + nb, sl].T

        kt = np.zeros((128, int(_KTOFF[-1])), np.float32)
        for ei, op in enumerate(_OPS):
            off = int(_KTOFF[ei])
            if op[0] == 'g':
                s0, sm, cols, lo_v, hi_v, binoff = _GROUPS[op[1]]
                p = off
                for s, M in sm:
                    wblock(kt[:, p : p + 2 * M], 8 * s + q, binoff, M)
                    p += 2 * M
                continue
            _, m, s, kind, cols, n0, n1 = op
            c = (M1C0 + 8 * s + q) if m == 1 else (8 * s + q)
            binoff = (128 if m == 1 else 0) + (64 if kind == 1 else 0)
            wblock(kt[:, off : off + cols], c, binoff, cols // 2)
        ktb = kt.astype(bf16)
        im = {
            f"kt{g}": np.ascontiguousarray(
                ktb[:, int(_KTOFF[_GSPL[g]]) : int(_KTOFF[_GSPL[g + 1]])]
            )
            for g in range(_NG)
        }
        im["xa"] = xa
        im["xb"] = xb
        in_maps.append(im)
    return in_maps


def _combine(outs):
    boff = np.cumsum([0] + _BANK_COLS)
    re_acc = np.zeros((KBINS, NF, NTRACKS), np.float32)
    im_acc = np.zeros((KBINS, NF, NTRACKS), np.float32)
    meta = {0: (0, 0), 1: (0, FH), 2: (64, 0), 3: (64, FH),
            4: (128, 0), 5: (128, FH), 6: (192, 0), 7: (192, FH)}
    for q in range(NCORES):
        o = np.asarray(outs[q]).astype(np.float32)
        for b in range(8):
            kb, fb = meta[b]
            nfr = _BANK_COLS[b] // 4
            nbins = min(64, KBINS - kb)
            blk = o[: 2 * nbins, int(boff[b]) : int(boff[b + 1])]
            blk = blk.reshape(2 * nbins, nfr, 4)
            re_acc[kb : kb + nbins, fb : fb + nfr] += blk[0::2]
            im_acc[kb : kb + nbins, fb : fb + nfr] += blk[1::2]
        for gi, (s0, sm, cols, lo_v, hi_v, binoff) in enumerate(_GROUPS):
            ln = hi_v - lo_v + 1
            rect = o[:cols, _GST[gi] : _GST[gi] + 4 * ln]
            rect = rect.reshape(cols, ln, 4)
            p = 0
            for s, M in sm:
                dv = 2 * (s - s0)
                a = max(0, lo_v - dv)
                b2 = min(NF - 1, hi_v - dv)
                sub = rect[p : p + 2 * M, a + dv - lo_v : b2 + dv - lo_v + 1]
                re_acc[binoff : binoff + M, a : b2 + 1] += sub[0::2]
                im_acc[binoff : binoff + M, a : b2 + 1] += sub[1::2]
                p += 2 * M
    y = np.sqrt(re_acc**2 + im_acc**2)  # [252, 129, 4]
    y = y.reshape(KBINS, NF, SR_B, SR_TR)
    return np.ascontiguousarray(y.transpose(2, 0, 1, 3))


def kernel(x, kr, ki):
    global _PROG
    from concourse.bass_utils import run_bass_kernel_spmd

    if _PROG is None:
        _PROG = _build_program()
    in_maps = _pack_inputs(x, kr, ki)
    res = run_bass_kernel_spmd(_PROG, in_maps, core_ids=list(range(NCORES)))
    outs = [res.results[q]["out"] for q in range(NCORES)]
    return _combine(outs)
